# revision 17
# baseline (speedup 1.0000x reference)
"""GAT-style masked self-attention (B=4, N=4096, D=128) on 8 trn2 NeuronCores.

reference:
    scores = X @ X^T / sqrt(D)            [B, N, N]
    masked = where(adj > 0, scores, -1e12)
    attn   = softmax(masked, axis=2)
    out    = attn @ X                     [B, N, D]

Sharding: 8 cores <- (batch b, row-half h); each core handles 2048 rows
of one batch element against all 4096 keys. No collectives.

Key order is rolled per core so the core's own rows are keys [0:R) --
softmax is key-order invariant, and this lets one SPMD program slice its
row block out of the same xt buffer on every core (no separate xtr DMA).

Device algorithm (per core), keys on partitions:
  - score matmul (fp16, full PE rate): psS = xt[:,k128].T @ xt[:,rows]
  - eviction+softmax work is split across TWO engines to balance the
    elementwise bottleneck (the old all-ACT eviction was the critical
    path at ~72us busy):
      * ACT supergroups: ACT evicts PSUM with exp fused (fp16), then DVE
        applies a multiplicative 0/1 fp8e4m3 mask at 2x rate.
      * DVE supergroups: ONE fused scalar_tensor_tensor per psS tile:
        u16 = uint16_sat(psS * A + madd), madd in {3584, -28672} fp8e5m2.
        The uint16 result IS the fp16 bit pattern of 2^(t/1024-15) ~
        exp(score*SCALE)*2^-11.5 (Schraudolph bit trick; the convert
        saturates negatives to 0, which both applies the mask and
        implements prob underflow). Sawtooth mean factor gbar is folded
        into the ACT path's exp bias so the two paths share one global
        scale that cancels in the softmax ratio.
  - AV matmul with the denominator fused via an appended ones-column:
      psO[rc] (+)= ptm[:, k, rc128].T @ [X_k | 1]   accumulated over k
      out = psO[:, :128] * (1 / psO[:, 128])
  - row blocks are software-pipelined: block i runs scores/evict while
    block i-1 runs its AV matmuls; AV matmuls are emitted first within
    each group so PE covers the eviction drain. The last two blocks are
    256 rows so the final (unoverlapped) AV drain is short.
"""

import math
import sys

sys.path.insert(0, "/opt/trn_rl_repo")

import numpy as np

B, N, D = 4, 4096, 128
R = N // 2            # rows per core
NK = N // 128         # 32 key tiles
SG = 8                # key tiles per super group (one mask DMA / mask op)
NSG = NK // SG
SCALE = 1.0 / math.sqrt(D)

# fast-exp bit trick constants (DVE supergroups)
A_TRICK = 1024.0 * math.log2(math.e) * SCALE      # 130.5778...
MADD_UNMASK = 3584.0                              # exact in fp8e5m2
MADD_MASK = -28672.0                              # exact in fp8e5m2
GBAR = 0.5 / math.log(2.0) ** 2                   # sawtooth mean 1.04068
# ACT path: exp(s*SCALE + EXP_BIAS) == GBAR * 2^(3584/1024 - 15) * exp(s*SCALE)
EXP_BIAS = -11.5 * math.log(2.0) + math.log(GBAR)  # -7.93131

# row blocks (offset, size): first and last two halved to shorten the
# (DMA-bound) pipeline fill and the AV drain tail
BLOCKS = [(0, 256), (256, 256), (512, 512), (1024, 512),
          (1536, 256), (1792, 256)]

CFG = dict(
    p_dt="float16",
    act_mask_dt="float16",    # multiplicative 0/1 mask for ACT supergroups
                              # (must be 2-byte: tensor_tensor 2x needs it)
    dve_mask_dt="float8e5",   # additive mask for DVE supergroups
    kg=2,                     # key tiles per PSUM score tile
    ptm_bufs=2,
    psum_s_bufs=2,
    adj_bufs=5,
    # (phase, sg) pairs evicted by the fused DVE trick; the rest go
    # through ACT exp. Chosen off the diagonal supergroups so the
    # dominant diagonal probs stay exact.
    dve_sgs=((0, 1), (1, 2), (2, 3), (3, 2), (4, 3), (5, 2)),
)

_CACHE = {}


def _mask_chunks(cfg):
    """Walk (phase, sg) in program order; assign flat column offsets in the
    per-kind mask tensors. Returns per-(phase,sg) records and totals."""
    dve = set(cfg["dve_sgs"])
    recs = {}
    tot = {"act": 0, "dve": 0}
    for phase, (off, bs) in enumerate(BLOCKS):
        for sg in range(NSG):
            kind = "dve" if (phase, sg) in dve else "act"
            recs[(phase, sg)] = (kind, tot[kind], SG * bs)
            tot[kind] += SG * bs
    return recs, tot


def _mask_jobs(cfg):
    """Group consecutive same-kind sgs of a phase into merged DMA jobs,
    capped at 16 KiB per partition (small dma_starts are latency-bound:
    each job costs ~2us of ring time regardless of size)."""
    recs, _ = _mask_chunks(cfg)
    esize = {"act": 2, "dve": 1}
    jobs_by_phase = []
    for phase, (off, bs) in enumerate(BLOCKS):
        jobs = []
        cur = None
        for sg in range(NSG):
            kind, c0, ncols = recs[(phase, sg)]
            nbytes = ncols * esize[kind]
            if (cur is not None and cur["kind"] == kind
                    and cur["bytes"] + nbytes <= 16384):
                cur["sgs"].append(sg)
                cur["ncols"] += ncols
                cur["bytes"] += nbytes
            else:
                cur = dict(kind=kind, c0=c0, ncols=ncols, bytes=nbytes,
                           sgs=[sg])
                jobs.append(cur)
        jobs_by_phase.append(jobs)
    return jobs_by_phase


def _build_nc(cfg):
    from concourse import bacc
    import concourse.mybir as mybir
    from concourse.tile import TileContext

    dt = mybir.dt
    p_dt = getattr(dt, cfg["p_dt"])
    am_dt = getattr(dt, cfg["act_mask_dt"])
    dm_dt = getattr(dt, cfg["dve_mask_dt"])
    kg = cfg["kg"]
    recs, tot = _mask_chunks(cfg)

    nc = bacc.Bacc(None, target_bir_lowering=False)

    xt_d = nc.dram_tensor("xt", [D, N], p_dt, kind="ExternalInput")
    # host pre-arranged to [128, NK, D+1] so the DMA is fully contiguous
    xaug_d = nc.dram_tensor("xaug", [128, NK, D + 1], p_dt,
                            kind="ExternalInput")
    adjm_d = nc.dram_tensor("adjm", [128, max(tot["act"], 1)], am_dt,
                            kind="ExternalInput")
    adja_d = nc.dram_tensor("adja", [128, max(tot["dve"], 1)], dm_dt,
                            kind="ExternalInput")
    o_d = nc.dram_tensor("o", [R, D], dt.float32, kind="ExternalOutput")

    with TileContext(nc) as tc:
        with (
            tc.tile_pool(name="singles", bufs=1) as singles,
            tc.tile_pool(name="ptm", bufs=cfg["ptm_bufs"]) as ptm_pool,
            tc.tile_pool(name="adj", bufs=cfg["adj_bufs"]) as adj_pool,
            tc.tile_pool(name="pe", bufs=3) as pe_pool,
            tc.tile_pool(name="outs", bufs=4) as out_pool,
            tc.tile_pool(name="small", bufs=4) as small_pool,
            tc.tile_pool(name="psS", bufs=cfg["psum_s_bufs"], space="PSUM") as psS_pool,
            tc.tile_pool(name="psO", bufs=4, space="PSUM") as psO_pool,
        ):
            ebias = singles.tile([128, 1], mybir.dt.float32)
            nc.vector.memset(ebias[:], EXP_BIAS)
            # warm the exp table while the init DMAs stream in
            warm = small_pool.tile([128, 1], mybir.dt.float32, tag="warm")
            nc.vector.memset(warm[:], 0.0)
            warm2 = small_pool.tile([128, 1], mybir.dt.float32, tag="warm")
            nc.scalar.activation(
                warm2[:], warm[:], mybir.ActivationFunctionType.Exp, scale=1.0
            )

            xt_sb = singles.tile([D, N], p_dt)
            xaug_sb = singles.tile([128, NK, D + 1], p_dt)

            jobs_by_phase = _mask_jobs(cfg)
            ring_bytes = {"sync": 0, "gpsimd": 0}

            def emit_jobs(phase):
                """DMA one phase's mask jobs, byte-balanced across the two
                DMA rings; returns per-sg (tile, index) views."""
                off, bs = BLOCKS[phase]
                views = [None] * NSG
                for j in jobs_by_phase[phase]:
                    kind, nsg = j["kind"], len(j["sgs"])
                    adt = am_dt if kind == "act" else dm_dt
                    src = adjm_d if kind == "act" else adja_d
                    t = adj_pool.tile([128, nsg, SG, bs], adt, tag="adj",
                                      name=f"adj_{phase}_{j['sgs'][0]}")
                    ring = min(ring_bytes, key=ring_bytes.get)
                    eng = nc.sync if ring == "sync" else nc.gpsimd
                    eng.dma_start(
                        out=t[:],
                        in_=src[:, j["c0"]:j["c0"] + j["ncols"]].rearrange(
                            "p (n k r) -> p n k r", n=nsg, k=SG),
                    )
                    ring_bytes[ring] += j["bytes"]
                    for i, sg in enumerate(j["sgs"]):
                        views[sg] = (t, i)
                return views

            # init DMAs staggered by first consumption: the first score
            # matmuls need xt keys (block-0 rows are keys [0:256) thanks
            # to the rolled order) plus block 0's mask chunks.
            nc.sync.dma_start(out=xt_sb[:, 0:1024], in_=xt_d[:, 0:1024])
            ring_bytes["sync"] += 2048
            adj_views0 = emit_jobs(0)
            nc.sync.dma_start(out=xt_sb[:, 1024:2048], in_=xt_d[:, 1024:2048])
            nc.sync.dma_start(out=xt_sb[:, 2048:4096], in_=xt_d[:, 2048:4096])
            ring_bytes["sync"] += 6144
            nc.gpsimd.dma_start(out=xaug_sb[:], in_=xaug_d[:, :, :])
            ring_bytes["gpsimd"] += 8256

            dve = set(cfg["dve_sgs"])
            NB = len(BLOCKS)
            ptm_prev = None
            bs_prev = None
            off_prev = None
            for phase in range(NB + 1):
                ptm_cur = None
                psO = None
                adj_views = None
                if phase < NB:
                    off, bs = BLOCKS[phase]
                    ptm_cur = ptm_pool.tile([128, NK, bs], p_dt, tag="ptm",
                                            name=f"ptm_{phase}")
                    adj_views = adj_views0 if phase == 0 else emit_jobs(phase)
                if phase >= 1:
                    psO = [
                        psO_pool.tile(
                            [128, D + 1], mybir.dt.float32,
                            tag="psO", name=f"psO_{phase}_{rc}",
                        )
                        for rc in range(bs_prev // 128)
                    ]

                if phase == NB:
                    # drain: rc-major AV bursts so each psO finishes early
                    # and its normalize/store overlaps the next burst
                    nrc = bs_prev // 128
                    o_sb = out_pool.tile([128, nrc, D], mybir.dt.float32,
                                         tag="o", name="od")
                    for rc in range(nrc):
                        for k in range(NK):
                            nc.tensor.matmul(
                                psO[rc][:, :],
                                lhsT=ptm_prev[:, k, rc * 128:(rc + 1) * 128],
                                rhs=xaug_sb[:, k, :],
                                start=(k == 0),
                                stop=(k == NK - 1),
                            )
                        recip = small_pool.tile([128, 1], mybir.dt.float32,
                                                tag="recip", name=f"recipd_{rc}")
                        nc.vector.reciprocal(recip[:], psO[rc][:, D:D + 1])
                        nc.vector.tensor_scalar_mul(o_sb[:, rc, :],
                                                    psO[rc][:, 0:D], recip[:])
                    nc.sync.dma_start(
                        out=o_d[off_prev:off_prev + bs_prev, :].rearrange(
                            "(c p) d -> p c d", p=128),
                        in_=o_sb[:],
                    )
                    break

                kg_b = kg * (512 // bs)   # keep kg_b*bs = 1024 elems per evict
                for sg in range(NSG):
                    is_dve = (phase, sg) in dve
                    k0 = sg * SG
                    pet = None
                    if not is_dve:
                        pet = pe_pool.tile([128, SG, bs], p_dt, tag="pe",
                                           name=f"pe_{phase}_{sg}")
                    for kgi in range(SG // kg_b):
                        # AV matmuls for the previous block first: PE has
                        # work while the evictions drain this group.
                        if phase >= 1:
                            for j in range(kg_b):
                                k = sg * SG + kgi * kg_b + j
                                for rc in range(bs_prev // 128):
                                    nc.tensor.matmul(
                                        psO[rc][:, :],
                                        lhsT=ptm_prev[:, k, rc * 128:(rc + 1) * 128],
                                        rhs=xaug_sb[:, k, :],
                                        start=(k == 0),
                                        stop=(k == NK - 1),
                                    )
                        ps = psS_pool.tile([128, kg_b, bs], mybir.dt.float32,
                                           tag="psS", name=f"psS_{phase}_{sg}_{kgi}")
                        for j in range(kg_b):
                            k = sg * SG + kgi * kg_b + j
                            nc.tensor.matmul(
                                ps[:, j, :],
                                lhsT=xt_sb[:, k * 128:(k + 1) * 128],
                                rhs=xt_sb[:, off:off + bs],
                                start=True,
                                stop=True,
                            )
                        at, ai = adj_views[sg]
                        if is_dve:
                            # fused evict+exp+mask: uint16 bit-trick, the
                            # saturating convert zeroes masked/underflowed
                            nc.vector.scalar_tensor_tensor(
                                ptm_cur[:, k0 + kgi * kg_b:
                                        k0 + (kgi + 1) * kg_b, :].bitcast(
                                            mybir.dt.uint16),
                                ps[:, :, :],
                                A_TRICK,
                                at[:, ai, kgi * kg_b:(kgi + 1) * kg_b, :],
                                op0=mybir.AluOpType.mult,
                                op1=mybir.AluOpType.add,
                            )
                        else:
                            nc.scalar.activation(
                                pet[:, kgi * kg_b:(kgi + 1) * kg_b, :],
                                ps[:, :, :],
                                mybir.ActivationFunctionType.Exp,
                                bias=ebias[:],
                                scale=SCALE,
                            )
                    if not is_dve:
                        at, ai = adj_views[sg]
                        nc.vector.tensor_mul(
                            ptm_cur[:, k0:k0 + SG, :],
                            pet[:, :, :],
                            at[:, ai, :, :],
                        )
                if phase >= 1:
                    nrc = bs_prev // 128
                    o_sb = out_pool.tile([128, nrc, D], mybir.dt.float32,
                                         tag="o", name=f"o_{phase}")
                    for rc in range(nrc):
                        recip = small_pool.tile([128, 1], mybir.dt.float32,
                                                tag="recip",
                                                name=f"recip_{phase}_{rc}")
                        nc.vector.reciprocal(recip[:], psO[rc][:, D:D + 1])
                        nc.vector.tensor_scalar_mul(o_sb[:, rc, :],
                                                    psO[rc][:, 0:D], recip[:])
                    nc.sync.dma_start(
                        out=o_d[off_prev:off_prev + bs_prev, :].rearrange(
                            "(c p) d -> p c d", p=128),
                        in_=o_sb[:],
                    )
                ptm_prev = ptm_cur
                bs_prev = bs
                off_prev = off
    nc.finalize()
    return nc


def _get_nc():
    key = str(sorted(CFG.items()))
    if key not in _CACHE:
        _CACHE[key] = _build_nc(CFG)
    return _CACHE[key]


def _np_dt(name):
    import ml_dtypes

    return {
        "float32": np.float32,
        "bfloat16": ml_dtypes.bfloat16,
        "float16": np.float16,
        "float8e4": ml_dtypes.float8_e4m3,
        "float8e5": ml_dtypes.float8_e5m2,
    }[name]


def make_in_maps(input, adj):
    """Host-side shard/layout prep: one input map per core."""
    input = np.asarray(input, dtype=np.float32)
    adj = np.asarray(adj)
    p_np = _np_dt(CFG["p_dt"])
    am_np = _np_dt(CFG["act_mask_dt"])
    dm_np = _np_dt(CFG["dve_mask_dt"])
    recs, tot = _mask_chunks(CFG)

    in_maps = []
    for core in range(8):
        b, h = core // 2, core % 2
        xb = input[b]                                    # [N, D]
        roll = np.concatenate([np.arange(h * R, N), np.arange(0, h * R)])
        xr = xb[roll]                                    # keys rolled
        xt = np.ascontiguousarray(xr.T).astype(p_np, copy=False)
        xaug = np.concatenate([xr, np.ones((N, 1), np.float32)], axis=1)
        # device layout [p=key%128, ktile, D+1], contiguous per partition
        xaug = np.ascontiguousarray(
            xaug.reshape(NK, 128, D + 1).transpose(1, 0, 2)
        ).astype(p_np)
        # mask[r, j] = adj[b][h*R + r, roll[j]] > 0; rows r are global
        mrows = adj[b][h * R:(h + 1) * R][:, roll] > 0   # [R, N] bool
        adjm = np.empty((128, max(tot["act"], 1)), am_np)
        adja = np.empty((128, max(tot["dve"], 1)), dm_np)
        for phase, (off, bs) in enumerate(BLOCKS):
            for sg in range(NSG):
                kind, c0, ncols = recs[(phase, sg)]
                sub = mrows[off:off + bs, sg * SG * 128:(sg + 1) * SG * 128]
                # [bs, SG, 128] -> [128(key%128), SG, bs] -> flat (SG, bs)
                subt = sub.reshape(bs, SG, 128).transpose(2, 1, 0)
                flat = subt.reshape(128, ncols)
                if kind == "act":
                    adjm[:, c0:c0 + ncols] = flat.astype(am_np)
                else:
                    adja[:, c0:c0 + ncols] = np.where(
                        flat, MADD_UNMASK, MADD_MASK
                    ).astype(dm_np)
        in_maps.append({"xt": xt, "xaug": xaug, "adjm": adjm, "adja": adja})
    return in_maps


def run_device(in_maps, trace=False, trace_cores=None):
    import concourse.bass_utils as bass_utils

    if trace:
        bass_utils.upload_artifacts = lambda tmpdir: ""  # no bucket in sandbox
    nc = _get_nc()
    return bass_utils.run_bass_kernel_spmd(
        nc, in_maps, list(range(8)), trace=trace, trace_cores=trace_cores
    )


def kernel(input, adj):
    res = run_device(make_in_maps(input, adj))
    out = np.empty((B, N, D), dtype=np.float32)
    for core in range(8):
        b, h = core // 2, core % 2
        out[b, h * R:(h + 1) * R, :] = res.results[core]["o"]
    return out


# revision 20
# speedup vs baseline: 1.0572x; 1.0572x over previous
"""GAT-style masked self-attention (B=4, N=4096, D=128) on 8 trn2 NeuronCores.

reference:
    scores = X @ X^T / sqrt(D)            [B, N, N]
    masked = where(adj > 0, scores, -1e12)
    attn   = softmax(masked, axis=2)
    out    = attn @ X                     [B, N, D]

Sharding: 8 cores <- (batch b, row-half h); each core handles 2048 rows
of one batch element against all 4096 keys. No collectives.

Key order is rolled per core so the core's own rows are keys [0:R) --
softmax is key-order invariant, and this lets one SPMD program slice its
row block out of the same xt buffer on every core (no separate xtr DMA).

Device algorithm (per core), keys on partitions:
  - score matmul (fp16, full PE rate): psS = xt[:,k128].T @ xt[:,rows]
  - eviction+softmax work is split across TWO engines to balance the
    elementwise bottleneck (the old all-ACT eviction was the critical
    path at ~72us busy):
      * ACT supergroups: ACT evicts PSUM with exp fused (fp16), then DVE
        applies a multiplicative 0/1 fp8e4m3 mask at 2x rate.
      * DVE supergroups: ONE fused scalar_tensor_tensor per psS tile:
        u16 = uint16_sat(psS * A + madd), madd in {3584, -28672} fp8e5m2.
        The uint16 result IS the fp16 bit pattern of 2^(t/1024-15) ~
        exp(score*SCALE)*2^-11.5 (Schraudolph bit trick; the convert
        saturates negatives to 0, which both applies the mask and
        implements prob underflow). Sawtooth mean factor gbar is folded
        into the ACT path's exp bias so the two paths share one global
        scale that cancels in the softmax ratio.
  - AV matmul with the denominator fused via an appended ones-column:
      psO[rc] (+)= ptm[:, k, rc128].T @ [X_k | 1]   accumulated over k
      out = psO[:, :128] * (1 / psO[:, 128])
  - row blocks are software-pipelined: block i runs scores/evict while
    block i-1 runs its AV matmuls; AV matmuls are emitted first within
    each group so PE covers the eviction drain. The last two blocks are
    256 rows so the final (unoverlapped) AV drain is short.
"""

import math
import sys

sys.path.insert(0, "/opt/trn_rl_repo")

import numpy as np

B, N, D = 4, 4096, 128
R = N // 2            # rows per core
NK = N // 128         # 32 key tiles
SG = 8                # key tiles per super group (one mask DMA / mask op)
NSG = NK // SG
SCALE = 1.0 / math.sqrt(D)

# fast-exp bit trick constants (DVE supergroups)
A_TRICK = 1024.0 * math.log2(math.e) * SCALE      # 130.5778...
MADD_UNMASK = 3584.0                              # exact in fp8e5m2
MADD_MASK = -28672.0                              # exact in fp8e5m2
GBAR = 0.5 / math.log(2.0) ** 2                   # sawtooth mean 1.04068
# ACT path: exp(s*SCALE + EXP_BIAS) == GBAR * 2^(3584/1024 - 15) * exp(s*SCALE)
EXP_BIAS = -11.5 * math.log(2.0) + math.log(GBAR)  # -7.93131

# row blocks (offset, size): first and last blocks small to shorten the
# (DMA-bound) pipeline fill and the AV drain tail
BLOCKS = [(0, 256), (256, 256), (512, 512), (1024, 512),
          (1536, 256), (1792, 128), (1920, 128)]

CFG = dict(
    p_dt="float16",
    act_mask_dt="float16",    # multiplicative 0/1 mask for ACT supergroups
                              # (must be 2-byte: tensor_tensor 2x needs it)
    dve_mask_dt="float8e5",   # additive mask for DVE supergroups
    kg=2,                     # key tiles per PSUM score tile
    ptm_bufs=2,
    psum_s_bufs=2,
    adj_bufs=5,
    # (phase, sg) pairs evicted by the fused DVE trick; the rest go
    # through ACT exp. Chosen off the diagonal supergroups so the
    # dominant diagonal probs stay exact.
    dve_sgs=((0, 1), (1, 2), (2, 3), (3, 2), (4, 3), (5, 2), (6, 3)),
)

_CACHE = {}


def _mask_chunks(cfg):
    """Walk (phase, sg) in program order; assign flat column offsets in the
    per-kind mask tensors. Returns per-(phase,sg) records and totals."""
    dve = set(cfg["dve_sgs"])
    recs = {}
    tot = {"act": 0, "dve": 0}
    for phase, (off, bs) in enumerate(BLOCKS):
        for sg in range(NSG):
            kind = "dve" if (phase, sg) in dve else "act"
            recs[(phase, sg)] = (kind, tot[kind], SG * bs)
            tot[kind] += SG * bs
    return recs, tot


def _mask_jobs(cfg):
    """Group consecutive same-kind sgs of a phase into merged DMA jobs,
    capped at 16 KiB per partition (small dma_starts are latency-bound:
    each job costs ~2us of ring time regardless of size)."""
    recs, _ = _mask_chunks(cfg)
    esize = {"act": 2, "dve": 1}
    jobs_by_phase = []
    for phase, (off, bs) in enumerate(BLOCKS):
        jobs = []
        cur = None
        for sg in range(NSG):
            kind, c0, ncols = recs[(phase, sg)]
            nbytes = ncols * esize[kind]
            if (cur is not None and cur["kind"] == kind
                    and cur["bytes"] + nbytes <= 16384):
                cur["sgs"].append(sg)
                cur["ncols"] += ncols
                cur["bytes"] += nbytes
            else:
                cur = dict(kind=kind, c0=c0, ncols=ncols, bytes=nbytes,
                           sgs=[sg])
                jobs.append(cur)
        jobs_by_phase.append(jobs)
    return jobs_by_phase


def _build_nc(cfg):
    from concourse import bacc
    import concourse.mybir as mybir
    from concourse.tile import TileContext

    dt = mybir.dt
    p_dt = getattr(dt, cfg["p_dt"])
    am_dt = getattr(dt, cfg["act_mask_dt"])
    dm_dt = getattr(dt, cfg["dve_mask_dt"])
    kg = cfg["kg"]
    recs, tot = _mask_chunks(cfg)

    nc = bacc.Bacc(None, target_bir_lowering=False)

    xt_d = nc.dram_tensor("xt", [D, N], p_dt, kind="ExternalInput")
    # host pre-arranged to [128, NK, D+1] so the DMA is fully contiguous
    xaug_d = nc.dram_tensor("xaug", [128, NK, D + 1], p_dt,
                            kind="ExternalInput")
    adjm_d = nc.dram_tensor("adjm", [128, max(tot["act"], 1)], am_dt,
                            kind="ExternalInput")
    adja_d = nc.dram_tensor("adja", [128, max(tot["dve"], 1)], dm_dt,
                            kind="ExternalInput")
    o_d = nc.dram_tensor("o", [R, D], dt.float32, kind="ExternalOutput")

    with TileContext(nc) as tc:
        with (
            tc.tile_pool(name="singles", bufs=1) as singles,
            tc.tile_pool(name="ptm", bufs=cfg["ptm_bufs"]) as ptm_pool,
            tc.tile_pool(name="adj", bufs=cfg["adj_bufs"]) as adj_pool,
            tc.tile_pool(name="pe", bufs=3) as pe_pool,
            tc.tile_pool(name="outs", bufs=4) as out_pool,
            tc.tile_pool(name="small", bufs=4) as small_pool,
            tc.tile_pool(name="psS", bufs=cfg["psum_s_bufs"], space="PSUM") as psS_pool,
            tc.tile_pool(name="psO", bufs=4, space="PSUM") as psO_pool,
        ):
            ebias = singles.tile([128, 1], mybir.dt.float32)
            nc.vector.memset(ebias[:], EXP_BIAS)
            # warm the exp table while the init DMAs stream in
            warm = small_pool.tile([128, 1], mybir.dt.float32, tag="warm")
            nc.vector.memset(warm[:], 0.0)
            warm2 = small_pool.tile([128, 1], mybir.dt.float32, tag="warm")
            nc.scalar.activation(
                warm2[:], warm[:], mybir.ActivationFunctionType.Exp, scale=1.0
            )

            xt_sb = singles.tile([D, N], p_dt)
            xaug_sb = singles.tile([128, NK, D + 1], p_dt)

            jobs_by_phase = _mask_jobs(cfg)
            ring_bytes = {"sync": 0, "gpsimd": 0}

            def emit_jobs(phase):
                """DMA one phase's mask jobs, byte-balanced across the two
                DMA rings; returns per-sg (tile, index) views."""
                off, bs = BLOCKS[phase]
                views = [None] * NSG
                for j in jobs_by_phase[phase]:
                    kind, nsg = j["kind"], len(j["sgs"])
                    adt = am_dt if kind == "act" else dm_dt
                    src = adjm_d if kind == "act" else adja_d
                    t = adj_pool.tile([128, nsg, SG, bs], adt, tag="adj",
                                      name=f"adj_{phase}_{j['sgs'][0]}")
                    ring = min(ring_bytes, key=ring_bytes.get)
                    eng = nc.sync if ring == "sync" else nc.gpsimd
                    eng.dma_start(
                        out=t[:],
                        in_=src[:, j["c0"]:j["c0"] + j["ncols"]].rearrange(
                            "p (n k r) -> p n k r", n=nsg, k=SG),
                    )
                    ring_bytes[ring] += j["bytes"]
                    for i, sg in enumerate(j["sgs"]):
                        views[sg] = (t, i)
                return views

            # init DMAs staggered by first consumption: the first score
            # matmuls need xt keys (block-0 rows are keys [0:256) thanks
            # to the rolled order), block 0's mask chunks, and the first
            # xaug k-tiles (block 0's AV sits ahead of block 1's scores
            # in the in-order PE queue, so late xaug stalls everything).
            nc.sync.dma_start(out=xt_sb[:, 0:512], in_=xt_d[:, 0:512])
            ring_bytes["sync"] += 1024
            nc.gpsimd.dma_start(out=xaug_sb[:, 0:8, :], in_=xaug_d[:, 0:8, :])
            ring_bytes["gpsimd"] += 8 * 258
            nc.sync.dma_start(out=xt_sb[:, 512:1024], in_=xt_d[:, 512:1024])
            ring_bytes["sync"] += 1024
            adj_views0 = emit_jobs(0)
            nc.sync.dma_start(out=xt_sb[:, 1024:2048], in_=xt_d[:, 1024:2048])
            nc.sync.dma_start(out=xt_sb[:, 2048:4096], in_=xt_d[:, 2048:4096])
            ring_bytes["sync"] += 6144
            nc.gpsimd.dma_start(out=xaug_sb[:, 8:NK, :],
                                in_=xaug_d[:, 8:NK, :])
            ring_bytes["gpsimd"] += 24 * 258

            dve = set(cfg["dve_sgs"])
            NB = len(BLOCKS)
            ptm_prev = None
            bs_prev = None
            off_prev = None
            for phase in range(NB + 1):
                ptm_cur = None
                psO = None
                adj_views = None
                if phase < NB:
                    off, bs = BLOCKS[phase]
                    ptm_cur = ptm_pool.tile([128, NK, bs], p_dt, tag="ptm",
                                            name=f"ptm_{phase}")
                    adj_views = adj_views0 if phase == 0 else emit_jobs(phase)
                if phase >= 1:
                    psO = [
                        psO_pool.tile(
                            [128, D + 1], mybir.dt.float32,
                            tag="psO", name=f"psO_{phase}_{rc}",
                        )
                        for rc in range(bs_prev // 128)
                    ]

                if phase == NB:
                    # drain: rc-major AV bursts so each psO finishes early
                    # and its normalize/store overlaps the next burst
                    nrc = bs_prev // 128
                    o_sb = out_pool.tile([128, nrc, D], mybir.dt.float32,
                                         tag="o", name="od")
                    for rc in range(nrc):
                        for k in range(NK):
                            nc.tensor.matmul(
                                psO[rc][:, :],
                                lhsT=ptm_prev[:, k, rc * 128:(rc + 1) * 128],
                                rhs=xaug_sb[:, k, :],
                                start=(k == 0),
                                stop=(k == NK - 1),
                            )
                        recip = small_pool.tile([128, 1], mybir.dt.float32,
                                                tag="recip", name=f"recipd_{rc}")
                        nc.vector.reciprocal(recip[:], psO[rc][:, D:D + 1])
                        nc.vector.tensor_scalar_mul(o_sb[:, rc, :],
                                                    psO[rc][:, 0:D], recip[:])
                    nc.sync.dma_start(
                        out=o_d[off_prev:off_prev + bs_prev, :].rearrange(
                            "(c p) d -> p c d", p=128),
                        in_=o_sb[:],
                    )
                    break

                kg_b = kg * (512 // bs)   # keep kg_b*bs = 1024 elems per evict
                for sg in range(NSG):
                    is_dve = (phase, sg) in dve
                    k0 = sg * SG
                    pet = None
                    if not is_dve:
                        pet = pe_pool.tile([128, SG, bs], p_dt, tag="pe",
                                           name=f"pe_{phase}_{sg}")
                    for kgi in range(SG // kg_b):
                        # AV matmuls for the previous block first: PE has
                        # work while the evictions drain this group.
                        if phase >= 1:
                            for j in range(kg_b):
                                k = sg * SG + kgi * kg_b + j
                                for rc in range(bs_prev // 128):
                                    nc.tensor.matmul(
                                        psO[rc][:, :],
                                        lhsT=ptm_prev[:, k, rc * 128:(rc + 1) * 128],
                                        rhs=xaug_sb[:, k, :],
                                        start=(k == 0),
                                        stop=(k == NK - 1),
                                    )
                        ps = psS_pool.tile([128, kg_b, bs], mybir.dt.float32,
                                           tag="psS", name=f"psS_{phase}_{sg}_{kgi}")
                        for j in range(kg_b):
                            k = sg * SG + kgi * kg_b + j
                            nc.tensor.matmul(
                                ps[:, j, :],
                                lhsT=xt_sb[:, k * 128:(k + 1) * 128],
                                rhs=xt_sb[:, off:off + bs],
                                start=True,
                                stop=True,
                            )
                        at, ai = adj_views[sg]
                        if is_dve:
                            # fused evict+exp+mask: uint16 bit-trick, the
                            # saturating convert zeroes masked/underflowed
                            nc.vector.scalar_tensor_tensor(
                                ptm_cur[:, k0 + kgi * kg_b:
                                        k0 + (kgi + 1) * kg_b, :].bitcast(
                                            mybir.dt.uint16),
                                ps[:, :, :],
                                A_TRICK,
                                at[:, ai, kgi * kg_b:(kgi + 1) * kg_b, :],
                                op0=mybir.AluOpType.mult,
                                op1=mybir.AluOpType.add,
                            )
                        else:
                            nc.scalar.activation(
                                pet[:, kgi * kg_b:(kgi + 1) * kg_b, :],
                                ps[:, :, :],
                                mybir.ActivationFunctionType.Exp,
                                bias=ebias[:],
                                scale=SCALE,
                            )
                    if not is_dve:
                        at, ai = adj_views[sg]
                        nc.vector.tensor_mul(
                            ptm_cur[:, k0:k0 + SG, :],
                            pet[:, :, :],
                            at[:, ai, :, :],
                        )
                if phase >= 1:
                    nrc = bs_prev // 128
                    o_sb = out_pool.tile([128, nrc, D], mybir.dt.float32,
                                         tag="o", name=f"o_{phase}")
                    for rc in range(nrc):
                        recip = small_pool.tile([128, 1], mybir.dt.float32,
                                                tag="recip",
                                                name=f"recip_{phase}_{rc}")
                        nc.vector.reciprocal(recip[:], psO[rc][:, D:D + 1])
                        nc.vector.tensor_scalar_mul(o_sb[:, rc, :],
                                                    psO[rc][:, 0:D], recip[:])
                    nc.sync.dma_start(
                        out=o_d[off_prev:off_prev + bs_prev, :].rearrange(
                            "(c p) d -> p c d", p=128),
                        in_=o_sb[:],
                    )
                ptm_prev = ptm_cur
                bs_prev = bs
                off_prev = off
    nc.finalize()
    return nc


def _get_nc():
    key = str(sorted(CFG.items()))
    if key not in _CACHE:
        _CACHE[key] = _build_nc(CFG)
    return _CACHE[key]


def _np_dt(name):
    import ml_dtypes

    return {
        "float32": np.float32,
        "bfloat16": ml_dtypes.bfloat16,
        "float16": np.float16,
        "float8e4": ml_dtypes.float8_e4m3,
        "float8e5": ml_dtypes.float8_e5m2,
    }[name]


def make_in_maps(input, adj):
    """Host-side shard/layout prep: one input map per core."""
    input = np.asarray(input, dtype=np.float32)
    adj = np.asarray(adj)
    p_np = _np_dt(CFG["p_dt"])
    am_np = _np_dt(CFG["act_mask_dt"])
    dm_np = _np_dt(CFG["dve_mask_dt"])
    recs, tot = _mask_chunks(CFG)

    in_maps = []
    for core in range(8):
        b, h = core // 2, core % 2
        xb = input[b]                                    # [N, D]
        roll = np.concatenate([np.arange(h * R, N), np.arange(0, h * R)])
        xr = xb[roll]                                    # keys rolled
        xt = np.ascontiguousarray(xr.T).astype(p_np, copy=False)
        xaug = np.concatenate([xr, np.ones((N, 1), np.float32)], axis=1)
        # device layout [p=key%128, ktile, D+1], contiguous per partition
        xaug = np.ascontiguousarray(
            xaug.reshape(NK, 128, D + 1).transpose(1, 0, 2)
        ).astype(p_np)
        # mask[r, j] = adj[b][h*R + r, roll[j]] > 0; rows r are global
        mrows = adj[b][h * R:(h + 1) * R][:, roll] > 0   # [R, N] bool
        adjm = np.empty((128, max(tot["act"], 1)), am_np)
        adja = np.empty((128, max(tot["dve"], 1)), dm_np)
        for phase, (off, bs) in enumerate(BLOCKS):
            for sg in range(NSG):
                kind, c0, ncols = recs[(phase, sg)]
                sub = mrows[off:off + bs, sg * SG * 128:(sg + 1) * SG * 128]
                # [bs, SG, 128] -> [128(key%128), SG, bs] -> flat (SG, bs)
                subt = sub.reshape(bs, SG, 128).transpose(2, 1, 0)
                flat = subt.reshape(128, ncols)
                if kind == "act":
                    adjm[:, c0:c0 + ncols] = flat.astype(am_np)
                else:
                    adja[:, c0:c0 + ncols] = np.where(
                        flat, MADD_UNMASK, MADD_MASK
                    ).astype(dm_np)
        in_maps.append({"xt": xt, "xaug": xaug, "adjm": adjm, "adja": adja})
    return in_maps


def run_device(in_maps, trace=False, trace_cores=None):
    import concourse.bass_utils as bass_utils

    if trace:
        bass_utils.upload_artifacts = lambda tmpdir: ""  # no bucket in sandbox
    nc = _get_nc()
    return bass_utils.run_bass_kernel_spmd(
        nc, in_maps, list(range(8)), trace=trace, trace_cores=trace_cores
    )


def kernel(input, adj):
    res = run_device(make_in_maps(input, adj))
    out = np.empty((B, N, D), dtype=np.float32)
    for core in range(8):
        b, h = core // 2, core % 2
        out[b, h * R:(h + 1) * R, :] = res.results[core]["o"]
    return out


# revision 22
# speedup vs baseline: 1.0746x; 1.0165x over previous
"""GAT-style masked self-attention (B=4, N=4096, D=128) on 8 trn2 NeuronCores.

reference:
    scores = X @ X^T / sqrt(D)            [B, N, N]
    masked = where(adj > 0, scores, -1e12)
    attn   = softmax(masked, axis=2)
    out    = attn @ X                     [B, N, D]

Sharding: 8 cores <- (batch b, row-half h); each core handles 2048 rows
of one batch element against all 4096 keys. No collectives.

Key order is rolled per core so the core's own rows are keys [0:R) --
softmax is key-order invariant, and this lets one SPMD program slice its
row block out of the same xt buffer on every core (no separate xtr DMA).

Device algorithm (per core), keys on partitions:
  - score matmul (fp16, full PE rate): psS = xt[:,k128].T @ xt[:,rows]
  - PSUM eviction + softmax is split across TWO engines, interleaved at
    eviction-group granularity (pattern A,A,A,D) so consecutive
    evictions alternate engines and overlap through the psS
    double-buffer (whole-supergroup assignment serialized the chain):
      * 'a' groups: ACT evicts with exp fused (fp16); DVE then applies a
        multiplicative 0/1 fp16 mask at 2x rate (one mask op per sg over
        the sg's act groups).
      * 'd' groups: ONE fused scalar_tensor_tensor per psS tile:
        u16 = uint16_sat(psS * A + madd), madd in {3584, -28672} fp8e5m2.
        The uint16 result IS the fp16 bit pattern of 2^(t/1024-15) ~
        exp(score*SCALE)*2^-11.5 (Schraudolph bit trick; the saturating
        convert zeroes masked/underflowed probs). The sawtooth mean
        factor gbar is folded into the ACT path's exp bias so both paths
        share one global scale that cancels in the softmax ratio.
  - AV matmul with the denominator fused via an appended ones-column:
      psO[rc] (+)= ptm[:, k, rc128].T @ [X_k | 1]   accumulated over k
      out = psO[:, :128] * (1 / psO[:, 128])
  - row blocks are software-pipelined: block i runs scores/evict while
    block i-1 runs its AV matmuls; AV matmuls are emitted first within
    each group so PE covers the eviction drain. First/last blocks are
    small to shorten the DMA-bound fill and the AV drain tail.
  - mask DMAs are merged into >=0.5MB jobs (small dma_starts are
    latency-bound: ~2us each) and byte-balanced across the sync and
    gpsimd DMA rings; xaug's first k-tiles ship first on gpsimd since
    block AV work sits ahead of the next block's scores in the in-order
    PE queue.
"""

import math
import sys

sys.path.insert(0, "/opt/trn_rl_repo")

import numpy as np

B, N, D = 4, 4096, 128
R = N // 2            # rows per core
NK = N // 128         # 32 key tiles
SG = 8                # key tiles per super group
NSG = NK // SG
SCALE = 1.0 / math.sqrt(D)
KG = 2                # key tiles per PSUM score tile at bs=512

# fast-exp bit trick constants ('d' eviction groups)
A_TRICK = 1024.0 * math.log2(math.e) * SCALE      # 130.5778...
MADD_UNMASK = 3584.0                              # exact in fp8e5m2
MADD_MASK = -28672.0                              # exact in fp8e5m2
GBAR = 0.5 / math.log(2.0) ** 2                   # sawtooth mean 1.04068
# ACT path: exp(s*SCALE + EXP_BIAS) == GBAR * 2^(3584/1024 - 15) * exp(s*SCALE)
EXP_BIAS = -11.5 * math.log(2.0) + math.log(GBAR)  # -7.93131

# row blocks (offset, size): first and last blocks small to shorten the
# (DMA-bound) pipeline fill and the AV drain tail
BLOCKS = [(0, 256), (256, 256), (512, 512), (1024, 512),
          (1536, 256), (1792, 128), (1920, 128)]

CFG = dict(
    p_dt="float16",
    act_mask_dt="float16",    # multiplicative 0/1 mask ('a' groups);
                              # must be 2-byte: tensor_tensor 2x needs it
    dve_mask_dt="float8e5",   # additive mask ('d' groups)
    ptm_bufs=2,
    psum_s_bufs=2,
    adj_bufs=5,
    # max mask-DMA job bytes per partition, per phase: small first jobs
    # cut the pipeline-fill latency (a job's completion semaphore fires
    # only when the whole job lands), big later jobs amortize the ~2us
    # per-job ring overhead
    job_caps=(4096, 8192, 16384, 16384, 16384, 16384, 16384),
)

_CACHE = {}


def _kg_b(bs):
    return KG * (512 // bs)   # key tiles per eviction group (1024 elems)


def _group_kinds(bs):
    """Eviction-group kinds per [sg][kgi]: ~1/4 'd' (fused DVE trick),
    placed at the end of each sg so every sg's 'a' key tiles stay
    contiguous, and interleaved so consecutive groups alternate engines."""
    gps = SG // _kg_b(bs)     # eviction groups per sg
    if gps == 4:
        return [["a", "a", "a", "d"]] * NSG
    if gps == 2:
        return [["a", "a"], ["a", "d"], ["a", "a"], ["a", "d"]]
    return [["a"], ["a"], ["a"], ["d"]]


def _mask_plan(cfg):
    """Flat column layout of the two mask tensors plus per-(phase,sg)
    views and merged DMA jobs."""
    esize = {"a": 2, "d": 1}
    tot = {"a": 0, "d": 0}
    sg_views = {}           # (phase, sg) -> dict(kind -> (c0, nkt))
    jobs_by_phase = []
    for phase, (off, bs) in enumerate(BLOCKS):
        kinds = _group_kinds(bs)
        kg_b = _kg_b(bs)
        jobs = []
        cur = {"a": None, "d": None}
        for sg in range(NSG):
            v = {}
            for kind in ("a", "d"):
                nkt = sum(kg_b for k in kinds[sg] if k == kind)
                if nkt == 0:
                    continue
                ncols = nkt * bs
                nbytes = ncols * esize[kind]
                v[kind] = (tot[kind], nkt)
                c = cur[kind]
                if c is not None and c["bytes"] + nbytes <= cfg["job_cap"]:
                    c["ncols"] += ncols
                    c["bytes"] += nbytes
                    c["sgs"].append(sg)
                else:
                    cur[kind] = dict(kind=kind, c0=tot[kind], ncols=ncols,
                                     bytes=nbytes, sgs=[sg])
                    jobs.append(cur[kind])
                tot[kind] += ncols
            sg_views[(phase, sg)] = v
        jobs_by_phase.append(jobs)
    return sg_views, tot, jobs_by_phase


def _build_nc(cfg):
    from concourse import bacc
    import concourse.mybir as mybir
    from concourse.tile import TileContext

    dt = mybir.dt
    p_dt = getattr(dt, cfg["p_dt"])
    am_dt = getattr(dt, cfg["act_mask_dt"])
    dm_dt = getattr(dt, cfg["dve_mask_dt"])
    sg_views, tot, jobs_by_phase = _mask_plan(cfg)

    nc = bacc.Bacc(None, target_bir_lowering=False)

    xt_d = nc.dram_tensor("xt", [D, N], p_dt, kind="ExternalInput")
    # host pre-arranged to [128, NK, D+1] so the DMA is fully contiguous
    xaug_d = nc.dram_tensor("xaug", [128, NK, D + 1], p_dt,
                            kind="ExternalInput")
    adjm_d = nc.dram_tensor("adjm", [128, max(tot["a"], 1)], am_dt,
                            kind="ExternalInput")
    adja_d = nc.dram_tensor("adja", [128, max(tot["d"], 1)], dm_dt,
                            kind="ExternalInput")
    o_d = nc.dram_tensor("o", [R, D], dt.float32, kind="ExternalOutput")

    with TileContext(nc) as tc:
        with (
            tc.tile_pool(name="singles", bufs=1) as singles,
            tc.tile_pool(name="ptm", bufs=cfg["ptm_bufs"]) as ptm_pool,
            tc.tile_pool(name="adj", bufs=cfg["adj_bufs"]) as adj_pool,
            tc.tile_pool(name="pe", bufs=3) as pe_pool,
            tc.tile_pool(name="outs", bufs=3) as out_pool,
            tc.tile_pool(name="small", bufs=4) as small_pool,
            tc.tile_pool(name="psS", bufs=cfg["psum_s_bufs"], space="PSUM") as psS_pool,
            tc.tile_pool(name="psO", bufs=4, space="PSUM") as psO_pool,
        ):
            ebias = singles.tile([128, 1], mybir.dt.float32)
            nc.vector.memset(ebias[:], EXP_BIAS)
            # warm the exp table while the init DMAs stream in
            warm = small_pool.tile([128, 1], mybir.dt.float32, tag="warm")
            nc.vector.memset(warm[:], 0.0)
            warm2 = small_pool.tile([128, 1], mybir.dt.float32, tag="warm")
            nc.scalar.activation(
                warm2[:], warm[:], mybir.ActivationFunctionType.Exp, scale=1.0
            )

            xt_sb = singles.tile([D, N], p_dt)
            xaug_sb = singles.tile([128, NK, D + 1], p_dt)

            ring_bytes = {"sync": 0, "gpsimd": 0}

            def emit_jobs(phase):
                """DMA one phase's mask jobs, byte-balanced across the
                two DMA rings; returns per-sg {kind: (tile, col0, nkt)}."""
                off, bs = BLOCKS[phase]
                tiles = {}
                for j in jobs_by_phase[phase]:
                    kind = j["kind"]
                    adt = am_dt if kind == "a" else dm_dt
                    src = adjm_d if kind == "a" else adja_d
                    t = adj_pool.tile([128, j["ncols"]], adt, tag="adj",
                                      name=f"adj{kind}_{phase}_{j['sgs'][0]}")
                    ring = min(ring_bytes, key=ring_bytes.get)
                    eng = nc.sync if ring == "sync" else nc.gpsimd
                    eng.dma_start(out=t[:],
                                  in_=src[:, j["c0"]:j["c0"] + j["ncols"]])
                    ring_bytes[ring] += j["bytes"]
                    for sg in j["sgs"]:
                        tiles[(sg, kind)] = (t, j["c0"])
                views = {}
                for sg in range(NSG):
                    v = {}
                    for kind, (c0, nkt) in sg_views[(phase, sg)].items():
                        t, jc0 = tiles[(sg, kind)]
                        v[kind] = (t, c0 - jc0, nkt)
                    views[sg] = v
                return views

            # init DMAs staggered by first consumption: the first score
            # matmuls need xt keys (block-0 rows are keys [0:256) thanks
            # to the rolled order), block 0's mask chunks, and the first
            # xaug k-tiles (block 0's AV sits ahead of block 1's scores
            # in the in-order PE queue, so late xaug stalls everything).
            nc.sync.dma_start(out=xt_sb[:, 0:512], in_=xt_d[:, 0:512])
            ring_bytes["sync"] += 1024
            nc.gpsimd.dma_start(out=xaug_sb[:, 0:8, :], in_=xaug_d[:, 0:8, :])
            ring_bytes["gpsimd"] += 8 * 258
            nc.sync.dma_start(out=xt_sb[:, 512:1024], in_=xt_d[:, 512:1024])
            ring_bytes["sync"] += 1024
            adj_views0 = emit_jobs(0)
            nc.sync.dma_start(out=xt_sb[:, 1024:2048], in_=xt_d[:, 1024:2048])
            nc.sync.dma_start(out=xt_sb[:, 2048:4096], in_=xt_d[:, 2048:4096])
            ring_bytes["sync"] += 6144
            nc.gpsimd.dma_start(out=xaug_sb[:, 8:NK, :],
                                in_=xaug_d[:, 8:NK, :])
            ring_bytes["gpsimd"] += 24 * 258

            NB = len(BLOCKS)
            ptm_prev = None
            bs_prev = None
            off_prev = None
            for phase in range(NB + 1):
                ptm_cur = None
                psO = None
                adj_views = None
                if phase < NB:
                    off, bs = BLOCKS[phase]
                    ptm_cur = ptm_pool.tile([128, NK, bs], p_dt, tag="ptm",
                                            name=f"ptm_{phase}")
                    adj_views = adj_views0 if phase == 0 else emit_jobs(phase)
                if phase >= 1:
                    psO = [
                        psO_pool.tile(
                            [128, D + 1], mybir.dt.float32,
                            tag="psO", name=f"psO_{phase}_{rc}",
                        )
                        for rc in range(bs_prev // 128)
                    ]

                if phase == NB:
                    # drain: rc-major AV bursts so each psO finishes early
                    # and its normalize overlaps the next burst
                    nrc = bs_prev // 128
                    o_sb = out_pool.tile([128, nrc, D], mybir.dt.float32,
                                         tag="o", name="od")
                    for rc in range(nrc):
                        for k in range(NK):
                            nc.tensor.matmul(
                                psO[rc][:, :],
                                lhsT=ptm_prev[:, k, rc * 128:(rc + 1) * 128],
                                rhs=xaug_sb[:, k, :],
                                start=(k == 0),
                                stop=(k == NK - 1),
                            )
                        recip = small_pool.tile([128, 1], mybir.dt.float32,
                                                tag="recip", name=f"recipd_{rc}")
                        nc.vector.reciprocal(recip[:], psO[rc][:, D:D + 1])
                        nc.vector.tensor_scalar_mul(o_sb[:, rc, :],
                                                    psO[rc][:, 0:D], recip[:])
                    nc.sync.dma_start(
                        out=o_d[off_prev:off_prev + bs_prev, :].rearrange(
                            "(c p) d -> p c d", p=128),
                        in_=o_sb[:],
                    )
                    break

                kg_b = _kg_b(bs)
                kinds = _group_kinds(bs)
                for sg in range(NSG):
                    k0 = sg * SG
                    n_act = sum(1 for k in kinds[sg] if k == "a")
                    pet = None
                    if n_act:
                        pet = pe_pool.tile([128, n_act * kg_b, bs], p_dt,
                                           tag="pe", name=f"pe_{phase}_{sg}")
                    nd = 0
                    for kgi in range(SG // kg_b):
                        # AV matmuls for the previous block first: PE has
                        # work while the evictions drain this group.
                        if phase >= 1:
                            for j in range(kg_b):
                                k = sg * SG + kgi * kg_b + j
                                for rc in range(bs_prev // 128):
                                    nc.tensor.matmul(
                                        psO[rc][:, :],
                                        lhsT=ptm_prev[:, k, rc * 128:(rc + 1) * 128],
                                        rhs=xaug_sb[:, k, :],
                                        start=(k == 0),
                                        stop=(k == NK - 1),
                                    )
                        ps = psS_pool.tile([128, kg_b, bs], mybir.dt.float32,
                                           tag="psS", name=f"psS_{phase}_{sg}_{kgi}")
                        for j in range(kg_b):
                            k = sg * SG + kgi * kg_b + j
                            nc.tensor.matmul(
                                ps[:, j, :],
                                lhsT=xt_sb[:, k * 128:(k + 1) * 128],
                                rhs=xt_sb[:, off:off + bs],
                                start=True,
                                stop=True,
                            )
                        if kinds[sg][kgi] == "d":
                            # fused evict+exp+mask: uint16 bit-trick; the
                            # saturating convert zeroes masked/underflowed
                            t, coff, nkt = adj_views[sg]["d"]
                            kd = k0 + kgi * kg_b
                            nc.vector.scalar_tensor_tensor(
                                ptm_cur[:, kd:kd + kg_b, :].bitcast(
                                    mybir.dt.uint16),
                                ps[:, :, :],
                                A_TRICK,
                                t[:, coff + nd * kg_b * bs:
                                  coff + (nd + 1) * kg_b * bs],
                                op0=mybir.AluOpType.mult,
                                op1=mybir.AluOpType.add,
                            )
                            nd += 1
                        else:
                            na = sum(1 for kk in kinds[sg][:kgi] if kk == "a")
                            nc.scalar.activation(
                                pet[:, na * kg_b:(na + 1) * kg_b, :],
                                ps[:, :, :],
                                mybir.ActivationFunctionType.Exp,
                                bias=ebias[:],
                                scale=SCALE,
                            )
                    if n_act:
                        t, coff, nkt = adj_views[sg]["a"]
                        nc.vector.tensor_mul(
                            ptm_cur[:, k0:k0 + n_act * kg_b, :],
                            pet[:, :, :],
                            t[:, coff:coff + nkt * bs],
                        )
                if phase >= 1:
                    nrc = bs_prev // 128
                    o_sb = out_pool.tile([128, nrc, D], mybir.dt.float32,
                                         tag="o", name=f"o_{phase}")
                    for rc in range(nrc):
                        recip = small_pool.tile([128, 1], mybir.dt.float32,
                                                tag="recip",
                                                name=f"recip_{phase}_{rc}")
                        nc.vector.reciprocal(recip[:], psO[rc][:, D:D + 1])
                        nc.vector.tensor_scalar_mul(o_sb[:, rc, :],
                                                    psO[rc][:, 0:D], recip[:])
                    nc.sync.dma_start(
                        out=o_d[off_prev:off_prev + bs_prev, :].rearrange(
                            "(c p) d -> p c d", p=128),
                        in_=o_sb[:],
                    )
                ptm_prev = ptm_cur
                bs_prev = bs
                off_prev = off
    nc.finalize()
    return nc


def _get_nc():
    key = str(sorted(CFG.items()))
    if key not in _CACHE:
        _CACHE[key] = _build_nc(CFG)
    return _CACHE[key]


def _np_dt(name):
    import ml_dtypes

    return {
        "float32": np.float32,
        "bfloat16": ml_dtypes.bfloat16,
        "float16": np.float16,
        "float8e4": ml_dtypes.float8_e4m3,
        "float8e5": ml_dtypes.float8_e5m2,
    }[name]


def make_in_maps(input, adj):
    """Host-side shard/layout prep: one input map per core."""
    input = np.asarray(input, dtype=np.float32)
    adj = np.asarray(adj)
    p_np = _np_dt(CFG["p_dt"])
    am_np = _np_dt(CFG["act_mask_dt"])
    dm_np = _np_dt(CFG["dve_mask_dt"])
    sg_views, tot, _ = _mask_plan(CFG)

    in_maps = []
    for core in range(8):
        b, h = core // 2, core % 2
        xb = input[b]                                    # [N, D]
        roll = np.concatenate([np.arange(h * R, N), np.arange(0, h * R)])
        xr = xb[roll]                                    # keys rolled
        xt = np.ascontiguousarray(xr.T).astype(p_np, copy=False)
        xaug = np.concatenate([xr, np.ones((N, 1), np.float32)], axis=1)
        # device layout [p=key%128, ktile, D+1], contiguous per partition
        xaug = np.ascontiguousarray(
            xaug.reshape(NK, 128, D + 1).transpose(1, 0, 2)
        ).astype(p_np)
        # mask[r, j] = adj[b][h*R + r, roll[j]] > 0
        mrows = adj[b][h * R:(h + 1) * R][:, roll] > 0   # [R, N] bool
        adjm = np.empty((128, max(tot["a"], 1)), am_np)
        adja = np.empty((128, max(tot["d"], 1)), dm_np)
        for phase, (off, bs) in enumerate(BLOCKS):
            for sg in range(NSG):
                kt_base = sg * SG
                kt_a = 0
                for kind, (c0, nkt) in sg_views[(phase, sg)].items():
                    if kind == "a":
                        kt0 = kt_base
                        kt_a = nkt
                    else:
                        kt0 = kt_base + kt_a
                    sub = mrows[off:off + bs,
                                kt0 * 128:(kt0 + nkt) * 128]
                    # [bs, nkt, 128] -> [128, nkt, bs] -> flat (nkt, bs)
                    flat = sub.reshape(bs, nkt, 128).transpose(2, 1, 0)
                    flat = flat.reshape(128, nkt * bs)
                    if kind == "a":
                        adjm[:, c0:c0 + nkt * bs] = flat.astype(am_np)
                    else:
                        adja[:, c0:c0 + nkt * bs] = np.where(
                            flat, MADD_UNMASK, MADD_MASK
                        ).astype(dm_np)
        in_maps.append({"xt": xt, "xaug": xaug, "adjm": adjm, "adja": adja})
    return in_maps


def run_device(in_maps, trace=False, trace_cores=None):
    import concourse.bass_utils as bass_utils

    if trace:
        bass_utils.upload_artifacts = lambda tmpdir: ""  # no bucket in sandbox
    nc = _get_nc()
    return bass_utils.run_bass_kernel_spmd(
        nc, in_maps, list(range(8)), trace=trace, trace_cores=trace_cores
    )


def kernel(input, adj):
    res = run_device(make_in_maps(input, adj))
    out = np.empty((B, N, D), dtype=np.float32)
    for core in range(8):
        b, h = core // 2, core % 2
        out[b, h * R:(h + 1) * R, :] = res.results[core]["o"]
    return out
